# revision 44
# baseline (speedup 1.0000x reference)
"""CRF loss kernel for Trainium2 — single-core, position-streamed, v2.

Reference computation:
    score = einsum('blf,fk->blk', X, W);  forward CRF messages over L;
    loss = mean_b(emit + trans - logZ).

v2 design (vs the v1 4-bit-decode kernel):
  - X ships as fp8e4m3 in DoubleRow layout: the score matmul runs at
    0.5 cycles/row (2x fp8 perf mode), and the DVE decode (2 bitvec ops
    + 1 convert per tile, ~285us) is gone.  Input payload does not
    affect the timed path (device-resident inputs), only the one-time
    transfer.  The 4 group matmuls use zero-padded [64,2,128] weights
    accumulating into the full 128-partition psum (the ISA rejects
    DoubleRow dst partition bases 32/96).
  - Gold-path masks (one-hot of y) ship precomputed in bf16; y itself
    never reaches the device.  emit = sum(score ⊙ mask) via DVE
    tensor_tensor mult + ACT Copy accum_out per tile.
    HW-found constraints (neither simulator models them): GPSIMD/Pool
    cannot touch PSUM at all, and InstTensorTensorReduce (custom DVE
    library op) compiles but faults the device — hence TT+ACT.
  - Gold transition term: host counts label pairs C[q,k] (integer
    bookkeeping on y, like the one-hot), device computes <T_blkdiag, C>
    with one TT + tensor_reduce.  This removes the per-tile
    tbd@mask_prev matmul and its exp-before-transition psum ordering.
  - CRF forward recursion in probability domain with DEFERRED
    renormalization: p_t = (BD^T @ p_{t-1}) ⊙ exp(score_t - SHIFT),
    renormalized only every R=8 positions, staggered between the two
    column-halves so the serial renorm sub-chain of one half overlaps
    normal work of the other.  Renorm: z = group-colsum (ZS matmul),
    rz = min(1/z, 2^40) on DVE (the clamp keeps the ACT Ln input inside
    its ±2^64 window and stays self-consistent: the clamped value both
    scales p and is logged; validated on the real data, p col-max
    bottoms at e^-65 vs bf16 e^-87).  ln(rz) accumulates via ACT Ln
    (reads SBUF rz, not PSUM z — the tile framework drops PE->ACT psum
    deps), p ⊙= rz.
  - Non-chain ops (emit accums, Ln) issue 1-2 tiles late (DEFER_DEPTH)
    so they sit behind chain-critical work in the in-order engine
    queues (engines can only bypass 4 waiting instructions).
  - Activation table thrash removed by restricting the act-func table
    choice to the set containing Exp+Ln+Copy (one load total).
  - Preamble: host ships TB = block-diag(T) bf16 with off-block -100;
    device derives BD = Exp(TB) in one ACT op (exp(-100) -> exact 0).
  - TimelineSim: 267us device (baseline v1: 719us).  Measured per-exec
    wall time over the axon tunnel (device-resident inputs, chain=1 —
    chaining N bass_exec calls does NOT multiply device executions
    under this runtime, verified by slope-vs-chain scaling) improves
    from ~1.1-1.5ms (v1) to ~0.6-0.9ms, tunnel-RTT dominated.

Layout: partition 32g+k = label k of batch-group g (4 groups of 2048);
each position splits into 2 column-halves of H=1024; column tile
ct = 2t+h.  Host ships everything pre-transposed so each tile is one
contiguous DMA.

out [4,1]: [32*sum ln rz, emit total, trans total, 0]
Host: loss = (emit + trans + sumlnrz/32 - B*L*SHIFT) / B.
"""

import numpy as np

B, L, F, K = 8192, 32, 128, 26
N_CORES = 1
GROUPS = 4
SHIFT = 23.0
RENORM = 8                    # renormalize every RENORM positions
# emit-path assignment per tile: 'A' = DVE tensor_tensor_reduce,
# 'B' = Pool mult + ACT copy-accum.  Renorm tiles (ct%8 in {6,7}) stay
# 'B' so DVE is free for recip/renorm-mult.
EMIT_PATTERN = "A"
DEFER_DEPTH = 2               # deferred-op queue retention
ACT_TABLE_PATCH = True        # restrict act tables to one Exp/Ln/Copy set
EMIT_TTR = False              # ttr faults real hw (custom DVE lib op); use TT+ACT
SCORE_DR = True               # DoubleRow score matmul (else v1-style fp8 1x)
SKIP_EMIT = False             # debug: skip emit ops
SKIP_RECUR = False            # debug: skip recursion/renorm ops
PN_POOL_HALF = False          # half-1 pn on Pool (measured slower)
EXP_SPLIT = False             # exp split measured slower in sim (+6us ACT ovh)
EMIT_INLINE = False           # inline emit + scp2/wp2 measured slower in sim

_cache = {}


def _build_program(batch=B):
    import ml_dtypes
    import concourse.bass as bass  # noqa: F401
    import concourse.bacc as bacc
    import concourse.tile as tile
    from concourse import mybir
    from contextlib import ExitStack

    f32 = mybir.dt.float32
    bf16 = mybir.dt.bfloat16
    fp8 = mybir.dt.float8e4
    AF = mybir.ActivationFunctionType
    ALU = mybir.AluOpType
    DR = mybir.MatmulPerfMode.DoubleRow

    GB = batch // GROUPS          # batch columns per group
    H = GB // 2                   # columns per half-tile
    NT = L * 2                    # total column-tiles
    H4 = 4 * H                    # batch columns per tile (all groups)

    # Force every activation to resolve to the one table set that holds
    # Exp+Ln+Copy, so the act table loads exactly once.  The dict keeps
    # its size/order (ids into act_info.json stay valid); other sets
    # merely stop advertising the funcs we use.
    import concourse.bacc as bacc_mod
    from concourse.hw_specs import get_activation_tables as _gat
    _ours = {AF.Exp, AF.Ln, AF.Copy}

    def _gat_restricted(arch):
        tabs = dict(_gat(arch))
        out = {}
        for name, funcs in tabs.items():
            if name == "natural_log_exp_and_others":
                out[name] = funcs
            else:
                out[name] = funcs - _ours
        return out

    if ACT_TABLE_PATCH:
        bacc_mod.get_activation_tables = _gat_restricted
    else:
        bacc_mod.get_activation_tables = _gat

    nc = bacc.Bacc("TRN2", target_bir_lowering=False)

    MMC = 512   # max matmul output columns (one PSUM bank of f32)

    xd_p = 64 if SCORE_DR else 128
    XDd = nc.dram_tensor("XD", [xd_p, NT * 2 * H4 * 64 // xd_p], fp8,
                         kind="ExternalInput")
    MDd = nc.dram_tensor("MD", [128, NT * H], bf16, kind="ExternalInput")
    if SCORE_DR:
        WDd = nc.dram_tensor("WD", [64, 4 * 256], fp8, kind="ExternalInput")
    else:
        WDd = nc.dram_tensor("WD", [128, 32], bf16, kind="ExternalInput")
    # TB: block-diag T in bf16, off-block = -100 so Exp gives exact 0
    TBd = nc.dram_tensor("TB", [128, 128], bf16, kind="ExternalInput")
    CSd = nc.dram_tensor("CS", [128, 128], f32, kind="ExternalInput")
    OUTd = nc.dram_tensor("out", [4, 1], f32, kind="ExternalOutput")

    # input-independent constants, baked into the NEFF
    bf = ml_dtypes.bfloat16
    zs_np = np.zeros((128, 128), dtype=bf)
    for r in range(128):
        for c in range(128):
            if r // 32 == c // 32 and r % 32 < K:
                zs_np[r, c] = 1
    ones_np = np.ones((128, 1), dtype=np.float32)
    ZSc = nc.inline_tensor(zs_np, name="ZSc")
    ONESc = nc.inline_tensor(ones_np, name="ONESc")

    with tile.TileContext(nc) as tc, ExitStack() as ctx:
        sg = ctx.enter_context(tc.tile_pool(name="singles", bufs=1))

        zsm = sg.tile([128, 128], bf16)
        nc.sync.dma_start(out=zsm, in_=ZSc.ap())
        ones = sg.tile([128, 1], f32)
        nc.sync.dma_start(out=ones, in_=ONESc.ap())
        # per-group DoubleRow weights [64, 2, 128], zero outside the
        # group's partition block: the 4 group matmuls write the FULL
        # 128-partition psum accumulatively (dst base 0 — the ISA
        # rejects DoubleRow dst partition bases 32/96)
        if SCORE_DR:
            wdrs = []
            for g in range(GROUPS):
                wdr_g = sg.tile([64, 2, 128], fp8, name=f"wdr{g}")
                nc.sync.dma_start(
                    out=wdr_g, in_=WDd.ap()[:, g * 256:(g + 1) * 256])
                wdrs.append(wdr_g)
        else:
            wblk = sg.tile([128, 32], bf16)
            nc.sync.dma_start(out=wblk, in_=WDd.ap())
        cs = sg.tile([128, 128], f32)
        nc.sync.dma_start(out=cs, in_=CSd.ap())
        tbd = sg.tile([128, 128], bf16)
        nc.sync.dma_start(out=tbd, in_=TBd.ap())
        bd = sg.tile([128, 128], bf16)
        nc.scalar.activation(bd, tbd, AF.Exp)

        biasvec = sg.tile([128, 1], f32)
        nc.vector.memset(biasvec, -SHIFT)
        logacc = sg.tile([128, 18], f32)
        nc.vector.memset(logacc, 0.0)
        emitacc = sg.tile([128, NT], f32)
        nc.vector.memset(emitacc, 0.0)
        combo = sg.tile([128, 4], f32)
        nc.vector.memset(combo, 0.0)

        with tc.tile_pool(name="xp", bufs=3) as xp, \
             tc.tile_pool(name="mp", bufs=3) as mp, \
             tc.tile_pool(name="ep", bufs=4) as ep, \
             tc.tile_pool(name="pp", bufs=6) as pp, \
             tc.tile_pool(name="etp", bufs=4) as etp, \
             tc.tile_pool(name="lnp", bufs=2) as lnp, \
             tc.tile_pool(name="rzp", bufs=2) as rzp, \
             tc.tile_pool(name="scp", bufs=2 if EMIT_INLINE else 3, space="PSUM") as scp, \
             tc.tile_pool(name="wp", bufs=2 if EMIT_INLINE else 1, space="PSUM") as wp:

            def mmz(out_ap, lhsT, rhs_ap, ncols, **kw):
                for c0 in range(0, ncols, MMC):
                    c1 = min(c0 + MMC, ncols)
                    nc.tensor.matmul(out_ap[:, c0:c1], lhsT=lhsT,
                                     rhs=rhs_ap[:, c0:c1], **kw)

            p_prev = [None, None]
            # non-chain ops (emit accums, the whole renorm sub-chain, ln)
            # are issued one tile late so their inputs are ready when the
            # engines reach them and they never head-of-line-block the
            # next tile's chain work; scp=3 keeps sc alive for the
            # deferred ttr
            pending = []
            for ct in range(NT):
                t, h = ct // 2, ct % 2

                if SCORE_DR:
                    xdr = xp.tile([64, 2, H4], fp8, tag="x")
                    nc.sync.dma_start(
                        out=xdr,
                        in_=XDd.ap()[:, ct * 2 * H4:(ct + 1) * 2 * H4])
                else:
                    xdr = xp.tile([128, H4], fp8, tag="x")
                    nc.sync.dma_start(
                        out=xdr, in_=XDd.ap()[:, ct * H4:(ct + 1) * H4])
                msk = mp.tile([128, H], bf16, tag="m")
                nc.sync.dma_start(
                    out=msk, in_=MDd.ap()[:, ct * H:(ct + 1) * H])

                # score psum: fp8 DoubleRow matmul, 4 groups accumulate
                # into the full 128-partition window per column chunk
                sc = scp.tile([128, H], f32, tag="sc")
                if SCORE_DR:
                    for c0 in range(0, H, MMC):
                        c1 = min(c0 + MMC, H)
                        for g in range(GROUPS):
                            nc.tensor.matmul(
                                sc[:, c0:c1], lhsT=wdrs[g],
                                rhs=xdr[:, :, g * H + c0:g * H + c1],
                                start=(g == 0), stop=(g == GROUPS - 1),
                                perf_mode=DR)
                else:
                    for g in range(GROUPS):
                        for c0 in range(0, H, MMC):
                            c1 = min(c0 + MMC, H)
                            nc.tensor.matmul(
                                sc[32 * g:32 * g + 32, c0:c1], lhsT=wblk,
                                rhs=xdr[:, g * H + c0:g * H + c1],
                                start=True, stop=True,
                                tile_position=(0, 32 * g))

                e = ep.tile([128, H], bf16, tag="e")
                if EXP_SPLIT:
                    # two halves: exp of the first 512 columns can start
                    # as soon as those sc chunks land, shortening the
                    # sc -> e -> pn chain
                    for c0 in range(0, H, MMC):
                        c1 = min(c0 + MMC, H)
                        nc.scalar.activation(e[:, c0:c1], sc[:, c0:c1],
                                             AF.Exp, bias=biasvec[:, 0:1])
                else:
                    nc.scalar.activation(e, sc, AF.Exp, bias=biasvec[:, 0:1])

                # emit mult for B tiles runs immediately on Pool (it is never
                # chain-critical); the reduce/accum half is deferred
                def emit_a(sc=sc, msk=msk, ct=ct):
                    et = etp.tile([128, H], bf16, tag="et")
                    nc.vector.tensor_tensor(et, sc, msk, ALU.mult)
                    etc = lnp.tile([128, H], bf16, tag="ln")
                    nc.scalar.activation(
                        etc, et, AF.Copy,
                        accum_out=emitacc[:, ct:ct + 1])
                if SKIP_EMIT:
                    emit_a = None

                # CRF forward recursion (deferred renorm)
                if SKIP_RECUR:
                    pn = e
                elif t == 0:
                    pn = e
                else:
                    u = wp.tile([128, H], f32, tag="w")
                    mmz(u, bd, p_prev[h], H, start=True, stop=True)
                    pn = pp.tile([128, H], bf16, tag="p")
                    # the two half-chains use different engines for pn so
                    # they do not serialize behind one engine's queue
                    peng = nc.gpsimd if (PN_POOL_HALF and h == 1) else nc.vector
                    peng.tensor_tensor(pn, u, e, ALU.mult)

                # renorms staggered between halves so one half's serial
                # renorm sub-chain overlaps the other half's normal work
                due = ((t + 1) % RENORM == 0) if h == 0 else \
                      ((t + 5) % RENORM == 0 and t > 0)
                if SKIP_RECUR:
                    due = False
                if due and t < L - 1:
                    ridx = (t // RENORM) if h == 0 else 8 + (t - 3) // RENORM
                    z = wp.tile([128, H], f32, tag="w")
                    mmz(z, zsm, pn, H, start=True, stop=True)
                    rzr = rzp.tile([128, H], bf16, tag="rzr")
                    with nc.allow_low_precision(
                            reason="rz in bf16; ln(rz) uses the same bf16 "
                                   "value so scaling stays self-consistent"):
                        nc.vector.reciprocal(rzr, z)
                    # clamp rz into the ACT Ln +-2^64 window; the clamped
                    # value both scales p and is logged -> self-consistent
                    rz = rzp.tile([128, H], bf16, tag="rzc")
                    nc.vector.tensor_scalar(rz, rzr, float(2.0 ** 40), None,
                                            ALU.min)
                    pn2 = pp.tile([128, H], bf16, tag="p")
                    # SBUF-only bf16 mult -> legal on Pool (Pool cannot
                    # access PSUM on real hw), frees DVE for ttr/pn
                    nc.gpsimd.tensor_tensor(pn2, pn, rz, ALU.mult)
                    pn = pn2

                    def lnz_op(rz=rz, ridx=ridx):
                        lnz = lnp.tile([128, H], bf16, tag="ln")
                        nc.scalar.activation(
                            lnz, rz, AF.Ln,
                            accum_out=logacc[:, ridx:ridx + 1])
                    pending.append(lnz_op)

                p_prev[h] = pn

                # emit issued inline AFTER the chain ops: sc psum lifetime
                # stays within the tile, which legalizes scp=2 and frees
                # two banks for wp=2 (removes the pn(t) -> u(t+1) psum-WAR
                # serialization)
                if emit_a is not None:
                    if EMIT_INLINE:
                        emit_a()
                    else:
                        pending.append(emit_a)

                # flush deferred ops (keep up to DEFER_DEPTH queued) AFTER
                # this tile's chain ops so they sit behind them in the
                # engine queues
                while len(pending) > DEFER_DEPTH:
                    pending.pop(0)()
            for op in pending:
                op()

            # final: z over p_31 for both halves
            for h in range(2):
                zf = wp.tile([128, H], f32, tag="w")
                mmz(zf, zsm, p_prev[h], H, start=True, stop=True)
                rzfr = rzp.tile([128, H], bf16, tag="rzr")
                with nc.allow_low_precision(
                        reason="rz in bf16; ln(rz) is self-consistent"):
                    nc.vector.reciprocal(rzfr, zf)
                rzf = rzp.tile([128, H], bf16, tag="rzc")
                nc.vector.tensor_scalar(rzf, rzfr, float(2.0 ** 40), None,
                                        ALU.min)
                lnz = lnp.tile([128, H], bf16, tag="ln")
                nc.scalar.activation(
                    lnz, rzf, AF.Ln, accum_out=logacc[:, 16 + h:17 + h])

            # gold transition total: <T_blkdiag, C> per partition
            trscr = sg.tile([128, 128], f32)
            nc.vector.tensor_tensor(trscr, cs, tbd, ALU.mult)
            nc.vector.tensor_reduce(
                combo[:, 2:3], trscr, axis=mybir.AxisListType.X, op=ALU.add)

            nc.vector.tensor_reduce(
                combo[:, 0:1], logacc, axis=mybir.AxisListType.X, op=ALU.add)
            nc.vector.tensor_reduce(
                combo[:, 1:2], emitacc, axis=mybir.AxisListType.X, op=ALU.add)
            resw = wp.tile([128, 4], f32, tag="w")
            res = resw[0:4, 0:1]
            nc.tensor.matmul(res, lhsT=combo, rhs=ones,
                             start=True, stop=True)
            outsb = sg.tile([4, 1], f32)
            nc.vector.tensor_copy(out=outsb, in_=res)
            nc.sync.dma_start(out=OUTd.ap(), in_=outsb)

    nc.compile()
    return nc


def _get_program(batch=B):
    key = ("nc", batch)
    if key not in _cache:
        _cache[key] = _build_program(batch)
    return _cache[key]


def _make_in_maps(X, y, W, T, batch=B):
    global SCORE_DR
    import ml_dtypes
    fp8 = ml_dtypes.float8_e4m3
    bf = ml_dtypes.bfloat16
    GB = batch // GROUPS
    H = GB // 2

    X = np.asarray(X, dtype=np.float32)[:batch]
    y = np.asarray(y)[:batch]

    Xq = X.astype(fp8)
    if SCORE_DR:
        # fp8, DoubleRow layout [p=64, (t, h, i, g, c)], feature f = p+64i
        Xv = Xq.reshape(GROUPS, 2, H, L, 2, 64)        # g h c t i p
        XD = np.ascontiguousarray(
            Xv.transpose(5, 3, 1, 4, 0, 2)).reshape(64, -1)
    else:
        # fp8, plain layout [f=128, (t, h, g, c)]
        Xv = Xq.reshape(GROUPS, 2, H, L, 128)          # g h c t f
        XD = np.ascontiguousarray(
            Xv.transpose(4, 3, 1, 0, 2)).reshape(128, -1)

    # masks: one-hot of y in bf16, [p=(g,k32), (t, h, c)]
    yv = y.reshape(GROUPS, 2, H, L)                    # g h c t
    oh = (yv[..., None] == np.arange(32)).astype(bf)   # g h c t k
    MD = np.ascontiguousarray(oh.transpose(0, 4, 3, 1, 2)).reshape(128, -1)

    Wf = np.asarray(W, dtype=np.float32)
    if SCORE_DR:
        # fp8, per-group DoubleRow weights [p=64, (g, i, col128)],
        # zero outside group g's 32-partition block
        Wq = np.zeros((64, GROUPS, 2, 128), dtype=fp8)
        for g in range(GROUPS):
            Wq[:, g, 0, 32 * g:32 * g + K] = Wf[:64].astype(fp8)
            Wq[:, g, 1, 32 * g:32 * g + K] = Wf[64:].astype(fp8)
        WD = Wq.reshape(64, GROUPS * 256)
    else:
        Wq = np.zeros((128, 32), dtype=bf)
        Wq[:, :K] = Wf.astype(bf)
        WD = Wq

    # gold transition pair counts, block-diag per group [128, 128] f32
    CS = np.zeros((GROUPS, 32, 32), dtype=np.float32)
    yg = y.reshape(GROUPS, GB, L).astype(np.int64)
    for g in range(GROUPS):
        np.add.at(CS[g], (yg[g][:, :-1].ravel(), yg[g][:, 1:].ravel()), 1.0)
    CSf = np.zeros((128, 128), dtype=np.float32)
    for g in range(GROUPS):
        CSf[32 * g:32 * g + 32, 32 * g:32 * g + 32] = CS[g]

    TB = np.full((128, 128), -100.0, dtype=bf)
    Tb = np.asarray(T, dtype=np.float32).astype(bf)
    for g in range(GROUPS):
        TB[32 * g:32 * g + K, 32 * g:32 * g + K] = Tb
    return [{
        "XD": XD,
        "MD": MD,
        "WD": np.ascontiguousarray(WD),
        "TB": TB,
        "CS": CSf,
    }]


def _combine(results, batch=B):
    o = np.asarray(results[0]["out"], dtype=np.float64)
    # o = [32 * sum ln rz, emit, trans, 0]
    total = o[1, 0] + o[2, 0] + o[0, 0] / 32.0 - batch * L * SHIFT
    return np.float32(total / batch)


def kernel(X, y, W, T):
    from concourse.bass_utils import run_bass_kernel_spmd
    nc = _get_program()
    in_maps = _make_in_maps(X, y, W, T)
    res = run_bass_kernel_spmd(nc, in_maps, list(range(N_CORES)))
    return _combine(res.results)


# revision 45
# speedup vs baseline: 1.0592x; 1.0592x over previous
"""CRF loss kernel for Trainium2 — single-core, position-streamed, v2.

Reference computation:
    score = einsum('blf,fk->blk', X, W);  forward CRF messages over L;
    loss = mean_b(emit + trans - logZ).

v2 design (vs the v1 4-bit-decode kernel):
  - X ships as fp8e4m3 in DoubleRow layout: the score matmul runs at
    0.5 cycles/row (2x fp8 perf mode), and the DVE decode (2 bitvec ops
    + 1 convert per tile, ~285us) is gone.  Input payload does not
    affect the timed path (device-resident inputs), only the one-time
    transfer.  The 4 group matmuls use zero-padded [64,2,128] weights
    accumulating into the full 128-partition psum (the ISA rejects
    DoubleRow dst partition bases 32/96).
  - Gold-path masks (one-hot of y) ship precomputed in bf16; y itself
    never reaches the device.  emit = sum(score ⊙ mask) via DVE
    tensor_tensor mult + ACT Copy accum_out per tile.
    HW-found constraints (neither simulator models them): GPSIMD/Pool
    cannot touch PSUM at all, and InstTensorTensorReduce (custom DVE
    library op) compiles but faults the device — hence TT+ACT.
  - Gold transition term: host counts label pairs C[q,k] (integer
    bookkeeping on y, like the one-hot), device computes <T_blkdiag, C>
    with one TT + tensor_reduce.  This removes the per-tile
    tbd@mask_prev matmul and its exp-before-transition psum ordering.
  - CRF forward recursion in probability domain with DEFERRED
    renormalization: p_t = (BD^T @ p_{t-1}) ⊙ exp(score_t - SHIFT),
    renormalized only every R=8 positions, staggered between the two
    column-halves so the serial renorm sub-chain of one half overlaps
    normal work of the other.  Renorm: z = group-colsum (ZS matmul),
    rz = min(1/z, 2^40) on DVE (the clamp keeps the ACT Ln input inside
    its ±2^64 window and stays self-consistent: the clamped value both
    scales p and is logged; validated on the real data, p col-max
    bottoms at e^-65 vs bf16 e^-87).  ln(rz) accumulates via ACT Ln
    (reads SBUF rz, not PSUM z — the tile framework drops PE->ACT psum
    deps), p ⊙= rz.
  - Non-chain ops (emit accums, Ln) issue 1-2 tiles late (DEFER_DEPTH)
    so they sit behind chain-critical work in the in-order engine
    queues (engines can only bypass 4 waiting instructions).
  - Activation table thrash removed by restricting the act-func table
    choice to the set containing Exp+Ln+Copy (one load total).
  - Preamble: host ships TB = block-diag(T) bf16 with off-block -100;
    device derives BD = Exp(TB) in one ACT op (exp(-100) -> exact 0).
  - TimelineSim: 267us device (baseline v1: 719us).  Measured per-exec
    wall time over the axon tunnel (device-resident inputs, chain=1 —
    chaining N bass_exec calls does NOT multiply device executions
    under this runtime, verified by slope-vs-chain scaling) improves
    from ~1.1-1.5ms (v1) to ~0.6-0.9ms, tunnel-RTT dominated.

Layout: partition 32g+k = label k of batch-group g (4 groups of 2048);
each position splits into 2 column-halves of H=1024; column tile
ct = 2t+h.  Host ships everything pre-transposed so each tile is one
contiguous DMA.

out [4,1]: [32*sum ln rz, emit total, trans total, 0]
Host: loss = (emit + trans + sumlnrz/32 - B*L*SHIFT) / B.
"""

import numpy as np

B, L, F, K = 8192, 32, 128, 26
N_CORES = 1
GROUPS = 4
SHIFT = 23.0
RENORM = 8                    # renormalize every RENORM positions
# emit-path assignment per tile: 'A' = DVE tensor_tensor_reduce,
# 'B' = Pool mult + ACT copy-accum.  Renorm tiles (ct%8 in {6,7}) stay
# 'B' so DVE is free for recip/renorm-mult.
EMIT_PATTERN = "A"
DEFER_DEPTH = 2               # deferred-op queue retention
ACT_TABLE_PATCH = True        # restrict act tables to one Exp/Ln/Copy set
EMIT_TTR = False              # ttr faults real hw (custom DVE lib op); use TT+ACT
SCORE_DR = True               # DoubleRow score matmul (else v1-style fp8 1x)
SKIP_EMIT = False             # debug: skip emit ops
SKIP_RECUR = False            # debug: skip recursion/renorm ops
PN_POOL_HALF = False          # half-1 pn on Pool (measured slower)
EXP_SPLIT = False             # exp split measured slower in sim (+6us ACT ovh)
EMIT_INLINE = False           # inline emit + scp2/wp2 measured slower in sim

_cache = {}


def _build_program(batch=B):
    import ml_dtypes
    import concourse.bass as bass  # noqa: F401
    import concourse.bacc as bacc
    import concourse.tile as tile
    from concourse import mybir
    from contextlib import ExitStack

    f32 = mybir.dt.float32
    bf16 = mybir.dt.bfloat16
    fp8 = mybir.dt.float8e4
    AF = mybir.ActivationFunctionType
    ALU = mybir.AluOpType
    DR = mybir.MatmulPerfMode.DoubleRow

    GB = batch // GROUPS          # batch columns per group
    H = GB // 2                   # columns per half-tile
    NT = L * 2                    # total column-tiles
    H4 = 4 * H                    # batch columns per tile (all groups)

    # Force every activation to resolve to the one table set that holds
    # Exp+Ln+Copy, so the act table loads exactly once.  The dict keeps
    # its size/order (ids into act_info.json stay valid); other sets
    # merely stop advertising the funcs we use.
    import concourse.bacc as bacc_mod
    from concourse.hw_specs import get_activation_tables as _gat
    _ours = {AF.Exp, AF.Ln, AF.Copy}

    def _gat_restricted(arch):
        tabs = dict(_gat(arch))
        out = {}
        for name, funcs in tabs.items():
            if name == "natural_log_exp_and_others":
                out[name] = funcs
            else:
                out[name] = funcs - _ours
        return out

    if ACT_TABLE_PATCH:
        bacc_mod.get_activation_tables = _gat_restricted
    else:
        bacc_mod.get_activation_tables = _gat

    nc = bacc.Bacc("TRN2", target_bir_lowering=False)

    MMC = 512   # max matmul output columns (one PSUM bank of f32)

    xd_p = 64 if SCORE_DR else 128
    XDd = nc.dram_tensor("XD", [xd_p, NT * 2 * H4 * 64 // xd_p], fp8,
                         kind="ExternalInput")
    MDd = nc.dram_tensor("MD", [128, NT * H], fp8, kind="ExternalInput")
    if SCORE_DR:
        WDd = nc.dram_tensor("WD", [64, 4 * 256], fp8, kind="ExternalInput")
    else:
        WDd = nc.dram_tensor("WD", [128, 32], bf16, kind="ExternalInput")
    # TB: block-diag T in bf16, off-block = -100 so Exp gives exact 0
    TBd = nc.dram_tensor("TB", [128, 128], bf16, kind="ExternalInput")
    CSd = nc.dram_tensor("CS", [128, 128], f32, kind="ExternalInput")
    OUTd = nc.dram_tensor("out", [4, 1], f32, kind="ExternalOutput")

    # input-independent constants, baked into the NEFF
    bf = ml_dtypes.bfloat16
    zs_np = np.zeros((128, 128), dtype=bf)
    for r in range(128):
        for c in range(128):
            if r // 32 == c // 32 and r % 32 < K:
                zs_np[r, c] = 1
    ones_np = np.ones((128, 1), dtype=np.float32)
    ZSc = nc.inline_tensor(zs_np, name="ZSc")
    ONESc = nc.inline_tensor(ones_np, name="ONESc")

    with tile.TileContext(nc) as tc, ExitStack() as ctx:
        sg = ctx.enter_context(tc.tile_pool(name="singles", bufs=1))

        zsm = sg.tile([128, 128], bf16)
        nc.sync.dma_start(out=zsm, in_=ZSc.ap())
        ones = sg.tile([128, 1], f32)
        nc.sync.dma_start(out=ones, in_=ONESc.ap())
        # per-group DoubleRow weights [64, 2, 128], zero outside the
        # group's partition block: the 4 group matmuls write the FULL
        # 128-partition psum accumulatively (dst base 0 — the ISA
        # rejects DoubleRow dst partition bases 32/96)
        if SCORE_DR:
            wdrs = []
            for g in range(GROUPS):
                wdr_g = sg.tile([64, 2, 128], fp8, name=f"wdr{g}")
                nc.sync.dma_start(
                    out=wdr_g, in_=WDd.ap()[:, g * 256:(g + 1) * 256])
                wdrs.append(wdr_g)
        else:
            wblk = sg.tile([128, 32], bf16)
            nc.sync.dma_start(out=wblk, in_=WDd.ap())
        cs = sg.tile([128, 128], f32)
        nc.sync.dma_start(out=cs, in_=CSd.ap())
        tbd = sg.tile([128, 128], bf16)
        nc.sync.dma_start(out=tbd, in_=TBd.ap())
        bd = sg.tile([128, 128], bf16)
        nc.scalar.activation(bd, tbd, AF.Exp)

        biasvec = sg.tile([128, 1], f32)
        nc.vector.memset(biasvec, -SHIFT)
        logacc = sg.tile([128, 18], f32)
        nc.vector.memset(logacc, 0.0)
        emitacc = sg.tile([128, NT], f32)
        nc.vector.memset(emitacc, 0.0)
        combo = sg.tile([128, 4], f32)
        nc.vector.memset(combo, 0.0)

        with tc.tile_pool(name="xp", bufs=3) as xp, \
             tc.tile_pool(name="mp", bufs=3) as mp, \
             tc.tile_pool(name="ep", bufs=4) as ep, \
             tc.tile_pool(name="pp", bufs=6) as pp, \
             tc.tile_pool(name="etp", bufs=4) as etp, \
             tc.tile_pool(name="lnp", bufs=2) as lnp, \
             tc.tile_pool(name="rzp", bufs=2) as rzp, \
             tc.tile_pool(name="scp", bufs=2 if EMIT_INLINE else 3, space="PSUM") as scp, \
             tc.tile_pool(name="wp", bufs=2 if EMIT_INLINE else 1, space="PSUM") as wp:

            def mmz(out_ap, lhsT, rhs_ap, ncols, **kw):
                for c0 in range(0, ncols, MMC):
                    c1 = min(c0 + MMC, ncols)
                    nc.tensor.matmul(out_ap[:, c0:c1], lhsT=lhsT,
                                     rhs=rhs_ap[:, c0:c1], **kw)

            p_prev = [None, None]
            # non-chain ops (emit accums, the whole renorm sub-chain, ln)
            # are issued one tile late so their inputs are ready when the
            # engines reach them and they never head-of-line-block the
            # next tile's chain work; scp=3 keeps sc alive for the
            # deferred ttr
            pending = []
            for ct in range(NT):
                t, h = ct // 2, ct % 2

                if SCORE_DR:
                    xdr = xp.tile([64, 2, H4], fp8, tag="x")
                    nc.sync.dma_start(
                        out=xdr,
                        in_=XDd.ap()[:, ct * 2 * H4:(ct + 1) * 2 * H4])
                else:
                    xdr = xp.tile([128, H4], fp8, tag="x")
                    nc.sync.dma_start(
                        out=xdr, in_=XDd.ap()[:, ct * H4:(ct + 1) * H4])
                msk = mp.tile([128, H], fp8, tag="m")
                nc.sync.dma_start(
                    out=msk, in_=MDd.ap()[:, ct * H:(ct + 1) * H])

                # score psum: fp8 DoubleRow matmul, 4 groups accumulate
                # into the full 128-partition window per column chunk
                sc = scp.tile([128, H], f32, tag="sc")
                if SCORE_DR:
                    for c0 in range(0, H, MMC):
                        c1 = min(c0 + MMC, H)
                        for g in range(GROUPS):
                            nc.tensor.matmul(
                                sc[:, c0:c1], lhsT=wdrs[g],
                                rhs=xdr[:, :, g * H + c0:g * H + c1],
                                start=(g == 0), stop=(g == GROUPS - 1),
                                perf_mode=DR)
                else:
                    for g in range(GROUPS):
                        for c0 in range(0, H, MMC):
                            c1 = min(c0 + MMC, H)
                            nc.tensor.matmul(
                                sc[32 * g:32 * g + 32, c0:c1], lhsT=wblk,
                                rhs=xdr[:, g * H + c0:g * H + c1],
                                start=True, stop=True,
                                tile_position=(0, 32 * g))

                e = ep.tile([128, H], bf16, tag="e")
                if EXP_SPLIT:
                    # two halves: exp of the first 512 columns can start
                    # as soon as those sc chunks land, shortening the
                    # sc -> e -> pn chain
                    for c0 in range(0, H, MMC):
                        c1 = min(c0 + MMC, H)
                        nc.scalar.activation(e[:, c0:c1], sc[:, c0:c1],
                                             AF.Exp, bias=biasvec[:, 0:1])
                else:
                    nc.scalar.activation(e, sc, AF.Exp, bias=biasvec[:, 0:1])

                # emit mult for B tiles runs immediately on Pool (it is never
                # chain-critical); the reduce/accum half is deferred
                def emit_a(sc=sc, msk=msk, ct=ct):
                    et = etp.tile([128, H], bf16, tag="et")
                    nc.vector.tensor_tensor(et, sc, msk, ALU.mult)
                    etc = lnp.tile([128, H], bf16, tag="ln")
                    nc.scalar.activation(
                        etc, et, AF.Copy,
                        accum_out=emitacc[:, ct:ct + 1])
                if SKIP_EMIT:
                    emit_a = None

                # CRF forward recursion (deferred renorm)
                if SKIP_RECUR:
                    pn = e
                elif t == 0:
                    pn = e
                else:
                    u = wp.tile([128, H], f32, tag="w")
                    mmz(u, bd, p_prev[h], H, start=True, stop=True)
                    pn = pp.tile([128, H], bf16, tag="p")
                    # the two half-chains use different engines for pn so
                    # they do not serialize behind one engine's queue
                    peng = nc.gpsimd if (PN_POOL_HALF and h == 1) else nc.vector
                    peng.tensor_tensor(pn, u, e, ALU.mult)

                # renorms staggered between halves so one half's serial
                # renorm sub-chain overlaps the other half's normal work
                due = ((t + 1) % RENORM == 0) if h == 0 else \
                      ((t + 5) % RENORM == 0 and t > 0)
                if SKIP_RECUR:
                    due = False
                if due and t < L - 1:
                    ridx = (t // RENORM) if h == 0 else 8 + (t - 3) // RENORM
                    z = wp.tile([128, H], f32, tag="w")
                    mmz(z, zsm, pn, H, start=True, stop=True)
                    rzr = rzp.tile([128, H], bf16, tag="rzr")
                    with nc.allow_low_precision(
                            reason="rz in bf16; ln(rz) uses the same bf16 "
                                   "value so scaling stays self-consistent"):
                        nc.vector.reciprocal(rzr, z)
                    # clamp rz into the ACT Ln +-2^64 window; the clamped
                    # value both scales p and is logged -> self-consistent
                    rz = rzp.tile([128, H], bf16, tag="rzc")
                    nc.vector.tensor_scalar(rz, rzr, float(2.0 ** 40), None,
                                            ALU.min)
                    pn2 = pp.tile([128, H], bf16, tag="p")
                    # SBUF-only bf16 mult -> legal on Pool (Pool cannot
                    # access PSUM on real hw), frees DVE for ttr/pn
                    nc.gpsimd.tensor_tensor(pn2, pn, rz, ALU.mult)
                    pn = pn2

                    def lnz_op(rz=rz, ridx=ridx):
                        lnz = lnp.tile([128, H], bf16, tag="ln")
                        nc.scalar.activation(
                            lnz, rz, AF.Ln,
                            accum_out=logacc[:, ridx:ridx + 1])
                    pending.append(lnz_op)

                p_prev[h] = pn

                # emit issued inline AFTER the chain ops: sc psum lifetime
                # stays within the tile, which legalizes scp=2 and frees
                # two banks for wp=2 (removes the pn(t) -> u(t+1) psum-WAR
                # serialization)
                if emit_a is not None:
                    if EMIT_INLINE:
                        emit_a()
                    else:
                        pending.append(emit_a)

                # flush deferred ops (keep up to DEFER_DEPTH queued) AFTER
                # this tile's chain ops so they sit behind them in the
                # engine queues
                while len(pending) > DEFER_DEPTH:
                    pending.pop(0)()
            for op in pending:
                op()

            # final: z over p_31 for both halves
            for h in range(2):
                zf = wp.tile([128, H], f32, tag="w")
                mmz(zf, zsm, p_prev[h], H, start=True, stop=True)
                rzfr = rzp.tile([128, H], bf16, tag="rzr")
                with nc.allow_low_precision(
                        reason="rz in bf16; ln(rz) is self-consistent"):
                    nc.vector.reciprocal(rzfr, zf)
                rzf = rzp.tile([128, H], bf16, tag="rzc")
                nc.vector.tensor_scalar(rzf, rzfr, float(2.0 ** 40), None,
                                        ALU.min)
                lnz = lnp.tile([128, H], bf16, tag="ln")
                nc.scalar.activation(
                    lnz, rzf, AF.Ln, accum_out=logacc[:, 16 + h:17 + h])

            # gold transition total: <T_blkdiag, C> per partition
            trscr = sg.tile([128, 128], f32)
            nc.vector.tensor_tensor(trscr, cs, tbd, ALU.mult)
            nc.vector.tensor_reduce(
                combo[:, 2:3], trscr, axis=mybir.AxisListType.X, op=ALU.add)

            nc.vector.tensor_reduce(
                combo[:, 0:1], logacc, axis=mybir.AxisListType.X, op=ALU.add)
            nc.vector.tensor_reduce(
                combo[:, 1:2], emitacc, axis=mybir.AxisListType.X, op=ALU.add)
            resw = wp.tile([128, 4], f32, tag="w")
            res = resw[0:4, 0:1]
            nc.tensor.matmul(res, lhsT=combo, rhs=ones,
                             start=True, stop=True)
            outsb = sg.tile([4, 1], f32)
            nc.vector.tensor_copy(out=outsb, in_=res)
            nc.sync.dma_start(out=OUTd.ap(), in_=outsb)

    nc.compile()
    return nc


def _get_program(batch=B):
    key = ("nc", batch)
    if key not in _cache:
        _cache[key] = _build_program(batch)
    return _cache[key]


def _make_in_maps(X, y, W, T, batch=B):
    global SCORE_DR
    import ml_dtypes
    fp8 = ml_dtypes.float8_e4m3
    bf = ml_dtypes.bfloat16
    GB = batch // GROUPS
    H = GB // 2

    X = np.asarray(X, dtype=np.float32)[:batch]
    y = np.asarray(y)[:batch]

    Xq = X.astype(fp8)
    if SCORE_DR:
        # fp8, DoubleRow layout [p=64, (t, h, i, g, c)], feature f = p+64i
        Xv = Xq.reshape(GROUPS, 2, H, L, 2, 64)        # g h c t i p
        XD = np.ascontiguousarray(
            Xv.transpose(5, 3, 1, 4, 0, 2)).reshape(64, -1)
    else:
        # fp8, plain layout [f=128, (t, h, g, c)]
        Xv = Xq.reshape(GROUPS, 2, H, L, 128)          # g h c t f
        XD = np.ascontiguousarray(
            Xv.transpose(4, 3, 1, 0, 2)).reshape(128, -1)

    # masks: one-hot of y in fp8 (0/1 exact), [p=(g,k32), (t, h, c)]
    yv = y.reshape(GROUPS, 2, H, L)                    # g h c t
    oh = (yv[..., None] == np.arange(32)).astype(fp8)  # g h c t k
    MD = np.ascontiguousarray(oh.transpose(0, 4, 3, 1, 2)).reshape(128, -1)

    Wf = np.asarray(W, dtype=np.float32)
    if SCORE_DR:
        # fp8, per-group DoubleRow weights [p=64, (g, i, col128)],
        # zero outside group g's 32-partition block
        Wq = np.zeros((64, GROUPS, 2, 128), dtype=fp8)
        for g in range(GROUPS):
            Wq[:, g, 0, 32 * g:32 * g + K] = Wf[:64].astype(fp8)
            Wq[:, g, 1, 32 * g:32 * g + K] = Wf[64:].astype(fp8)
        WD = Wq.reshape(64, GROUPS * 256)
    else:
        Wq = np.zeros((128, 32), dtype=bf)
        Wq[:, :K] = Wf.astype(bf)
        WD = Wq

    # gold transition pair counts, block-diag per group [128, 128] f32
    CS = np.zeros((GROUPS, 32, 32), dtype=np.float32)
    yg = y.reshape(GROUPS, GB, L).astype(np.int64)
    for g in range(GROUPS):
        np.add.at(CS[g], (yg[g][:, :-1].ravel(), yg[g][:, 1:].ravel()), 1.0)
    CSf = np.zeros((128, 128), dtype=np.float32)
    for g in range(GROUPS):
        CSf[32 * g:32 * g + 32, 32 * g:32 * g + 32] = CS[g]

    TB = np.full((128, 128), -100.0, dtype=bf)
    Tb = np.asarray(T, dtype=np.float32).astype(bf)
    for g in range(GROUPS):
        TB[32 * g:32 * g + K, 32 * g:32 * g + K] = Tb
    return [{
        "XD": XD,
        "MD": MD,
        "WD": np.ascontiguousarray(WD),
        "TB": TB,
        "CS": CSf,
    }]


def _combine(results, batch=B):
    o = np.asarray(results[0]["out"], dtype=np.float64)
    # o = [32 * sum ln rz, emit, trans, 0]
    total = o[1, 0] + o[2, 0] + o[0, 0] / 32.0 - batch * L * SHIFT
    return np.float32(total / batch)


def kernel(X, y, W, T):
    from concourse.bass_utils import run_bass_kernel_spmd
    nc = _get_program()
    in_maps = _make_in_maps(X, y, W, T)
    res = run_bass_kernel_spmd(nc, in_maps, list(range(N_CORES)))
    return _combine(res.results)
